# revision 7
# baseline (speedup 1.0000x reference)
"""AffinityLoss (segment-reduce) Trainium2 kernel.

Math (single pass over the data — no per-row center gather needed):
    lbl     = argmax(labels, axis=1)                         (N,)
    sums_c  = sum of features rows with lbl == c             (C, D)
    n_c     = count of rows with lbl == c                    (C,)
    sumsq   = sum(features ** 2)                             scalar
    centers = where(n>0, sums/max(n,1), 0) + 1e-6
    intra   = sumsq - 2*sum(sums*centers) + sum(n_c*||c_c||^2)
    inter   = sum((centers - mean(centers))^2) / C
    loss    = intra / (inter + 1e-6)

Per core (data-parallel over N):
  - one-hot(argmax) built on the vector engine (reduce_max + one
    broadcast is_equal over the whole supertile)
  - segment sums via PE: one matmul per 128-row group
    (one-hot^T @ features) accumulated in PSUM over the full loop
  - counts via PE with ones as the stationary operand; ALL counts
    matmuls (every j-group of every supertile) accumulate into a single
    [1, cc*C] PSUM tile (accumulation commutes), which closes at the
    last MAIN supertile so the counts copy + DMA-out happen mid-stream
  - sum-of-squares on the scalar engine (Square activation + accumulate)

Features stream as f32 -> bf16 cast DMAs (SWDGE), contiguous per
partition per supertile; the supertile schedule tapers at the end so the
compute tail after the last DMA is short.  Labels run `lead` supertiles
ahead of the feature stream (and the tail supertiles' labels + one-hots
+ counts matmuls are prepared even earlier), so every one-hot is ready
before its features arrive: after the final feature DMA only the last
sums matmuls + the [C,D] PSUM readout remain.  That readout is split in
two halves whose DMA issues run on the sync and scalar sequencers in
parallel; the scalar engine's stream stays pure Squares so the final
sqacc column is ready right behind the last feature tile.  The O(C*D)
finalization runs on the host over the 8 per-core partials (the
gather/unshard step).

Roofline: 46.66MB/core over 16 DMA queues at ~26GB/s each ≈ 111.5us of
pure wire time; ~6us of fixed preamble.
"""

import numpy as np

import concourse.bacc as bacc
import concourse.tile as tile
from concourse import mybir
from concourse.bass_utils import run_bass_kernel_spmd

N_CORES = 8
N_TOTAL = 262144
D = 256
C = 100
P = 128
T = 16  # 128-row groups per supertile (DMA batch)

F32 = mybir.dt.float32
BF16 = mybir.dt.bfloat16


def build_nc(
    rows_per_core: int,
    t: int = T,
    bufs: int = 6,
    lead: int = 3,
    tail_prep_ahead: int = 4,
):
    """Build the per-core Bass program (same SPMD program on all cores).

    lead: how many supertiles ahead of the feature stream the main
    supertiles' labels are issued.  tail_prep_ahead: how many iterations
    before the end of the main stream the tail supertiles' labels /
    one-hots / counts matmuls are emitted.
    """
    total_j = rows_per_core // P
    cc = 4  # j's per counts matmul (free dim cc*C <= 512)
    assert t % cc == 0
    # Supertile schedule: mostly t, tapering at the end so the compute tail
    # after the final DMA is short.
    if t % 8 == 0:
        tail = [t // 2, t // 4, t // 8, t // 8]
    else:
        tail = [t // 2, t // 2]
    if total_j > 2 * t and t >= 8 and (total_j - sum(tail)) % t == 0:
        sched = [t] * ((total_j - sum(tail)) // t) + tail
        n_main = len(sched) - len(tail)
    else:
        assert total_j % t == 0
        sched = [t] * (total_j // t)
        n_main = len(sched)
    assert sum(sched) == total_j
    n_super = len(sched)
    lead = min(lead, n_main - 1)
    have_tail = n_main < n_super
    # tail counts must land before the counts-stop at iter n_main-1
    prep_iter = min(max(lead, n_main - tail_prep_ahead), n_main - 2)
    assert not have_tail or prep_iter < n_main - 1

    nc = bacc.Bacc(
        "TRN2", target_bir_lowering=False, debug=False, num_devices=N_CORES
    )

    feats = nc.dram_tensor(
        "features", [rows_per_core, D], F32, kind="ExternalInput"
    ).ap()
    labels = nc.dram_tensor(
        "labels", [rows_per_core, C], F32, kind="ExternalInput"
    ).ap()
    out_partial = nc.dram_tensor(
        "partial", [C, D], F32, kind="ExternalOutput"
    ).ap()
    out_counts = nc.dram_tensor(
        "counts", [1, cc * C], F32, kind="ExternalOutput"
    ).ap()
    out_sqacc = nc.dram_tensor(
        "sqacc", [P, n_super], F32, kind="ExternalOutput"
    ).ap()

    # Blocked row mapping per supertile: row = row0 + p*ts + j -> partition p
    # reads ts contiguous rows (one contiguous DRAM chunk per partition).

    with tile.TileContext(nc) as tc:
        with (
            tc.tile_pool(name="feat", bufs=bufs) as feat_pool,
            tc.tile_pool(name="lbl", bufs=4) as lbl_pool,
            tc.tile_pool(name="oh", bufs=lead + 2) as oh_pool,
            tc.tile_pool(name="sq", bufs=2) as sq_pool,
            tc.tile_pool(name="acc", bufs=1) as acc_pool,
            tc.tile_pool(name="ps", bufs=1, space="PSUM") as psum_pool,
        ):
            psum_sums = psum_pool.tile([C, D], F32, tag="ps_sums")
            psum_cnt = psum_pool.tile([1, cc * C], F32, tag="ps_cnt")
            sqacc = acc_pool.tile([P, n_super], F32, tag="sqacc")
            ones = acc_pool.tile([P, 1], BF16, tag="ones")
            part_sb = acc_pool.tile([C, D], F32, tag="part")
            cnt_sb = acc_pool.tile([1, cc * C], F32, tag="cnt")
            nc.vector.memset(ones[:, :], 1.0)

            def make_onehot(lbl_ap, mx, oh, ts):
                nc.vector.reduce_max(
                    mx[:, :ts], lbl_ap, axis=mybir.AxisListType.X
                )
                mxb = mx[:, :ts].unsqueeze(-1).broadcast_to((P, ts, C))
                nc.vector.tensor_tensor(
                    out=oh[:, :ts, :], in0=lbl_ap, in1=mxb,
                    op=mybir.AluOpType.is_equal,
                )

            def cnt_matmul(oh, j0, w, start, stop):
                # ones^T @ onehot[:, j0:j0+w] -> per-(j,c) column counts,
                # accumulated into the single shared counts PSUM region.
                nc.tensor.matmul(
                    psum_cnt[:, : w * C],
                    ones[:, :],
                    oh[:, j0 : j0 + w],
                    start=start,
                    stop=stop,
                )

            row_start = [P * sum(sched[:i]) for i in range(n_super)]
            onehots = {}

            def emit_onehot(i):
                # labels for main supertile i: DMA + one-hot on the DVE,
                # `lead` supertiles before i's features arrive.
                ts = sched[i]
                r0 = row_start[i]
                lv = labels[r0 : r0 + P * ts].rearrange(
                    "(p j) c -> p j c", p=P, j=ts
                )
                lbl_t = lbl_pool.tile([P, t, C], F32, tag="lbl")
                nc.sync.dma_start(out=lbl_t[:, :ts, :], in_=lv)
                mx = oh_pool.tile([P, t], F32, tag="mx")
                oh = oh_pool.tile([P, t, C], BF16, tag="oh")
                make_onehot(lbl_t[:, :ts, :], mx, oh, ts)
                onehots[i] = oh

            def emit_tail_prep():
                # tail supertiles: labels + one-hots into persistent tiles,
                # plus their counts matmuls (which only need the one-hot),
                # all well before the end of the stream.
                for i in range(n_main, n_super):
                    ts = sched[i]
                    r0 = row_start[i]
                    lv = labels[r0 : r0 + P * ts].rearrange(
                        "(p j) c -> p j c", p=P, j=ts
                    )
                    lt = acc_pool.tile(
                        [P, ts, C], F32, tag=f"lblt{i}", name=f"lblt{i}"
                    )
                    mxt = acc_pool.tile(
                        [P, ts], F32, tag=f"mxt{i}", name=f"mxt{i}"
                    )
                    oht = acc_pool.tile(
                        [P, ts, C], BF16, tag=f"oht{i}", name=f"oht{i}"
                    )
                    nc.sync.dma_start(out=lt[:, :, :], in_=lv)
                    make_onehot(lt[:, :, :], mxt, oht, ts)
                    onehots[i] = oht
                    for g in range(0, ts, cc):
                        w = min(cc, ts - g)
                        cnt_matmul(oht, g, w, start=False, stop=False)

            for i in range(lead):
                emit_onehot(i)

            for s, ts in enumerate(sched):
                fv = feats[row_start[s] : row_start[s] + P * ts].rearrange(
                    "(p j) d -> p j d", p=P, j=ts
                )
                feat_t = feat_pool.tile([P, t, D], BF16, tag="feat")
                # SWDGE (gpsimd) casts f32 -> bf16 during the transfer
                nc.gpsimd.dma_start(out=feat_t[:, :ts, :], in_=fv)

                if s + lead < n_main:
                    emit_onehot(s + lead)
                onehot = onehots.pop(s)

                sq_t = sq_pool.tile([P, t, D], BF16, tag="sq")
                nc.scalar.activation(
                    sq_t[:, :ts, :],
                    feat_t[:, :ts, :],
                    mybir.ActivationFunctionType.Square,
                    accum_out=sqacc[:, s : s + 1],
                )

                for j in range(ts):
                    nc.tensor.matmul(
                        psum_sums[:, :],
                        onehot[:, j],
                        feat_t[:, j],
                        start=(s == 0 and j == 0),
                        stop=(s == n_super - 1 and j == ts - 1),
                    )
                # counts for MAIN supertiles run inline; all tail counts
                # were emitted at prep_iter, so the shared PSUM region
                # closes here at the last main supertile -> ship early.
                if s < n_main:
                    for g in range(0, ts, cc):
                        w = min(cc, ts - g)
                        cnt_matmul(
                            onehot,
                            g,
                            w,
                            start=(s == 0 and g == 0),
                            stop=(s == n_main - 1 and g + cc >= ts),
                        )
                    if s == n_main - 1:
                        nc.vector.tensor_copy(cnt_sb[:, :], psum_cnt[:, :])
                        nc.sync.dma_start(
                            out=out_counts[:, :], in_=cnt_sb[:, :]
                        )
                if have_tail and s == prep_iter:
                    emit_tail_prep()

            # Final readout: two halves, copies on vector, DMA issues on
            # sync + scalar in parallel; sqacc ships once at the end (its
            # producer chain -- pure Squares on scalar -- is never blocked).
            h = D // 2
            nc.vector.tensor_copy(part_sb[:, :h], psum_sums[:, :h])
            nc.sync.dma_start(out=out_partial[:, :h], in_=part_sb[:, :h])
            nc.vector.tensor_copy(part_sb[:, h:], psum_sums[:, h:])
            nc.scalar.dma_start(out=out_partial[:, h:], in_=part_sb[:, h:])
            nc.sync.dma_start(out=out_sqacc[:, :], in_=sqacc[:, :])

    nc.compile()
    return nc


_NC_CACHE: dict = {}


def _get_nc():
    if "nc" not in _NC_CACHE:
        _NC_CACHE["nc"] = build_nc(N_TOTAL // N_CORES)
    return _NC_CACHE["nc"]


def finalize(partials, countss, sqaccs):
    """Host gather/unshard: combine per-core partials into the scalar loss."""
    sums = np.zeros((C, D), np.float64)
    counts = np.zeros((C,), np.float64)
    sumsq = 0.0
    for part, cnt, sq in zip(partials, countss, sqaccs):
        sums += part.astype(np.float64).reshape(C, -1, D).sum(axis=1)
        counts += cnt.astype(np.float64).reshape(-1, C).sum(axis=0)
        sumsq += float(sq.astype(np.float64).sum())
    centers = (
        np.where(counts[:, None] > 0, sums / np.maximum(counts, 1.0)[:, None], 0.0)
        + 1e-6
    )
    intra = (
        sumsq
        - 2.0 * float((sums * centers).sum())
        + float((counts * (centers**2).sum(axis=1)).sum())
    )
    cmean = centers.mean(axis=0, keepdims=True)
    inter = float(((centers - cmean) ** 2).sum()) / C
    loss = intra / (inter + 1e-6)
    return np.array(loss, dtype=np.float32)


def kernel(features: np.ndarray, labels: np.ndarray) -> np.ndarray:
    features = np.asarray(features)
    labels = np.asarray(labels)
    assert features.shape == (N_TOTAL, D), features.shape
    assert labels.shape == (N_TOTAL, C), labels.shape
    nc = _get_nc()
    rows = N_TOTAL // N_CORES
    in_maps = []
    for i in range(N_CORES):
        sl = slice(i * rows, (i + 1) * rows)
        in_maps.append(
            {
                "features": np.ascontiguousarray(features[sl], dtype=np.float32),
                "labels": np.ascontiguousarray(labels[sl], dtype=np.float32),
            }
        )
    res = run_bass_kernel_spmd(nc, in_maps, list(range(N_CORES)))
    return finalize(
        [r["partial"] for r in res.results],
        [r["counts"] for r in res.results],
        [r["sqacc"] for r in res.results],
    )


# revision 11
# speedup vs baseline: 1.1636x; 1.1636x over previous
"""AffinityLoss (segment-reduce) Trainium2 kernel.

Math (single pass over the data — no per-row center gather needed):
    lbl     = argmax(labels, axis=1)                         (N,)
    sums_c  = sum of features rows with lbl == c             (C, D)
    n_c     = count of rows with lbl == c                    (C,)
    sumsq   = sum(features ** 2)                             scalar
    centers = where(n>0, sums/max(n,1), 0) + 1e-6
    intra   = sumsq - 2*sum(sums*centers) + sum(n_c*||c_c||^2)
    inter   = sum((centers - mean(centers))^2) / C
    loss    = intra / (inter + 1e-6)

Per core (data-parallel over N):
  - one-hot(argmax) built on the vector engine (reduce_max + one
    broadcast is_equal over the whole supertile)
  - segment sums via PE: one matmul per 128-row group
    (one-hot^T @ features) accumulated in PSUM over the full loop
  - counts via PE with ones as the stationary operand; ALL counts
    matmuls (every j-group of every supertile) accumulate into a single
    [1, cc*C] PSUM tile (accumulation commutes), which closes at the
    last MAIN supertile so the counts copy + DMA-out happen mid-stream
  - sum-of-squares on the scalar engine (Square activation + accumulate)

Features stream as f32 -> bf16 cast DMAs (SWDGE), contiguous per
partition per supertile; the supertile schedule tapers at the end so the
compute tail after the last DMA is short.  Labels run `lead` supertiles
ahead of the feature stream (and the tail supertiles' labels + one-hots
+ counts matmuls are prepared even earlier), so every one-hot is ready
before its features arrive: after the final feature DMA only the last
sums matmuls + the [C,D] PSUM readout remain.  That readout is split in
two halves whose DMA issues run on the sync and scalar sequencers in
parallel; the scalar engine's stream stays pure Squares so the final
sqacc column is ready right behind the last feature tile.  The O(C*D)
finalization runs on the host over the 8 per-core partials (the
gather/unshard step).

Roofline: 46.66MB/core over 16 DMA queues at ~26GB/s each ≈ 111.5us of
pure wire time; ~6us of fixed preamble.
"""

import numpy as np

import concourse.bacc as bacc
import concourse.tile as tile
from concourse import mybir
from concourse.bass_utils import run_bass_kernel_spmd

N_CORES = 8
N_TOTAL = 262144
D = 256
C = 100
P = 128
T = 16  # 128-row groups per supertile (DMA batch)

F32 = mybir.dt.float32
BF16 = mybir.dt.bfloat16


def build_nc(
    rows_per_core: int,
    t: int = T,
    bufs: int = 6,
    lead: int = 3,
    tail_prep_ahead: int = 4,
):
    """Build the per-core Bass program (same SPMD program on all cores).

    lead: how many supertiles ahead of the feature stream the main
    supertiles' labels are issued.  tail_prep_ahead: how many iterations
    before the end of the main stream the tail supertiles' labels /
    one-hots / counts matmuls are emitted.
    """
    total_j = rows_per_core // P
    cc = 4  # j's per counts matmul (free dim cc*C <= 512)
    assert t % cc == 0
    # Supertile schedule: mostly t, tapering at the end so the compute tail
    # after the final DMA is short.
    if t % 8 == 0:
        tail = [t // 2, t // 4, t // 8, t // 8]
    else:
        tail = [t // 2, t // 2]
    if total_j > 2 * t and t >= 8 and (total_j - sum(tail)) % t == 0:
        sched = [t] * ((total_j - sum(tail)) // t) + tail
        n_main = len(sched) - len(tail)
    else:
        assert total_j % t == 0
        sched = [t] * (total_j // t)
        n_main = len(sched)
    assert sum(sched) == total_j
    n_super = len(sched)
    lead = min(lead, n_main - 1)
    have_tail = n_main < n_super
    # Tail labels are issued very early: DMA-queue arrival order follows
    # issue order, so a late issue would land behind several supertiles of
    # already-issued feature descriptors and arrive only at the very end.
    # The tail COUNTS matmuls are emitted a few iterations later so the
    # in-order PE reaches them well after the tail one-hots are ready
    # (a too-early rendezvous would stall the PE and, once the feature
    # pool fills, the whole stream).
    tl_iter = min(2, n_main - 2)
    tc_iter = min(7, n_main - 2)
    assert not have_tail or tl_iter <= tc_iter < n_main - 1

    nc = bacc.Bacc(
        "TRN2", target_bir_lowering=False, debug=False, num_devices=N_CORES
    )

    feats = nc.dram_tensor(
        "features", [rows_per_core, D], F32, kind="ExternalInput"
    ).ap()
    labels = nc.dram_tensor(
        "labels", [rows_per_core, C], F32, kind="ExternalInput"
    ).ap()
    out_partial = nc.dram_tensor(
        "partial", [C, D], F32, kind="ExternalOutput"
    ).ap()
    out_counts = nc.dram_tensor(
        "counts", [1, cc * C], F32, kind="ExternalOutput"
    ).ap()
    out_sqacc = nc.dram_tensor(
        "sqacc", [P, n_super], F32, kind="ExternalOutput"
    ).ap()

    # Blocked row mapping per supertile: row = row0 + p*ts + j -> partition p
    # reads ts contiguous rows (one contiguous DRAM chunk per partition).

    with tile.TileContext(nc) as tc:
        with (
            tc.tile_pool(name="feat", bufs=bufs) as feat_pool,
            tc.tile_pool(name="lbl", bufs=4) as lbl_pool,
            tc.tile_pool(name="oh", bufs=lead + 2) as oh_pool,
            tc.tile_pool(name="sq", bufs=2) as sq_pool,
            tc.tile_pool(name="acc", bufs=1) as acc_pool,
            tc.tile_pool(name="ps", bufs=1, space="PSUM") as psum_pool,
        ):
            psum_sums = psum_pool.tile([C, D], F32, tag="ps_sums")
            psum_cnt = psum_pool.tile([1, cc * C], F32, tag="ps_cnt")
            sqacc = acc_pool.tile([P, n_super], F32, tag="sqacc")
            ones = acc_pool.tile([P, 1], BF16, tag="ones")
            part_sb = acc_pool.tile([C, D], F32, tag="part")
            cnt_sb = acc_pool.tile([1, cc * C], F32, tag="cnt")
            nc.vector.memset(ones[:, :], 1.0)

            def make_onehot(lbl_ap, mx, oh, ts):
                nc.vector.reduce_max(
                    mx[:, :ts], lbl_ap, axis=mybir.AxisListType.X
                )
                mxb = mx[:, :ts].unsqueeze(-1).broadcast_to((P, ts, C))
                nc.vector.tensor_tensor(
                    out=oh[:, :ts, :], in0=lbl_ap, in1=mxb,
                    op=mybir.AluOpType.is_equal,
                )

            def cnt_matmul(oh, j0, w, start, stop):
                # ones^T @ onehot[:, j0:j0+w] -> per-(j,c) column counts,
                # accumulated into the single shared counts PSUM region.
                nc.tensor.matmul(
                    psum_cnt[:, : w * C],
                    ones[:, :],
                    oh[:, j0 : j0 + w],
                    start=start,
                    stop=stop,
                )

            row_start = [P * sum(sched[:i]) for i in range(n_super)]
            onehots = {}

            def emit_onehot(i):
                # labels for main supertile i: DMA + one-hot on the DVE,
                # `lead` supertiles before i's features arrive.
                ts = sched[i]
                r0 = row_start[i]
                lv = labels[r0 : r0 + P * ts].rearrange(
                    "(p j) c -> p j c", p=P, j=ts
                )
                lbl_t = lbl_pool.tile([P, t, C], F32, tag="lbl")
                nc.sync.dma_start(out=lbl_t[:, :ts, :], in_=lv)
                mx = oh_pool.tile([P, t], F32, tag="mx")
                oh = oh_pool.tile([P, t, C], BF16, tag="oh")
                make_onehot(lbl_t[:, :ts, :], mx, oh, ts)
                onehots[i] = oh

            def emit_tail_labels():
                # tail supertiles: labels + one-hots into persistent tiles
                for i in range(n_main, n_super):
                    ts = sched[i]
                    r0 = row_start[i]
                    lv = labels[r0 : r0 + P * ts].rearrange(
                        "(p j) c -> p j c", p=P, j=ts
                    )
                    lt = acc_pool.tile(
                        [P, ts, C], F32, tag=f"lblt{i}", name=f"lblt{i}"
                    )
                    mxt = acc_pool.tile(
                        [P, ts], F32, tag=f"mxt{i}", name=f"mxt{i}"
                    )
                    oht = acc_pool.tile(
                        [P, ts, C], BF16, tag=f"oht{i}", name=f"oht{i}"
                    )
                    nc.sync.dma_start(out=lt[:, :, :], in_=lv)
                    make_onehot(lt[:, :, :], mxt, oht, ts)
                    onehots[i] = oht

            def emit_tail_cnts():
                for i in range(n_main, n_super):
                    ts = sched[i]
                    for g in range(0, ts, cc):
                        w = min(cc, ts - g)
                        cnt_matmul(onehots[i], g, w, start=False, stop=False)

            for i in range(lead):
                emit_onehot(i)

            for s, ts in enumerate(sched):
                fv = feats[row_start[s] : row_start[s] + P * ts].rearrange(
                    "(p j) d -> p j d", p=P, j=ts
                )
                feat_t = feat_pool.tile([P, t, D], BF16, tag="feat")
                # SWDGE (gpsimd) casts f32 -> bf16 during the transfer
                nc.gpsimd.dma_start(out=feat_t[:, :ts, :], in_=fv)

                if s + lead < n_main:
                    emit_onehot(s + lead)
                onehot = onehots.pop(s)

                sq_t = sq_pool.tile([P, t, D], BF16, tag="sq")
                nc.scalar.activation(
                    sq_t[:, :ts, :],
                    feat_t[:, :ts, :],
                    mybir.ActivationFunctionType.Square,
                    accum_out=sqacc[:, s : s + 1],
                )

                for j in range(ts):
                    nc.tensor.matmul(
                        psum_sums[:, :],
                        onehot[:, j],
                        feat_t[:, j],
                        start=(s == 0 and j == 0),
                        stop=(s == n_super - 1 and j == ts - 1),
                    )
                # counts for MAIN supertiles run inline; all tail counts
                # were emitted at prep_iter, so the shared PSUM region
                # closes here at the last main supertile -> ship early.
                if s < n_main:
                    for g in range(0, ts, cc):
                        w = min(cc, ts - g)
                        cnt_matmul(
                            onehot,
                            g,
                            w,
                            start=(s == 0 and g == 0),
                            stop=(s == n_main - 1 and g + cc >= ts),
                        )
                    if s == n_main - 1:
                        nc.vector.tensor_copy(cnt_sb[:, :], psum_cnt[:, :])
                        nc.sync.dma_start(
                            out=out_counts[:, :], in_=cnt_sb[:, :]
                        )
                if have_tail and s == tl_iter:
                    emit_tail_labels()
                if have_tail and s == tc_iter:
                    emit_tail_cnts()

            # Final readout: two halves, copies on vector, DMA issues on
            # sync + scalar in parallel; sqacc ships once at the end (its
            # producer chain -- pure Squares on scalar -- is never blocked).
            h = D // 2
            nc.vector.tensor_copy(part_sb[:, :h], psum_sums[:, :h])
            nc.sync.dma_start(out=out_partial[:, :h], in_=part_sb[:, :h])
            nc.vector.tensor_copy(part_sb[:, h:], psum_sums[:, h:])
            nc.scalar.dma_start(out=out_partial[:, h:], in_=part_sb[:, h:])
            nc.gpsimd.dma_start(out=out_sqacc[:, :], in_=sqacc[:, :])

    nc.compile()
    return nc


_NC_CACHE: dict = {}


def _get_nc():
    if "nc" not in _NC_CACHE:
        _NC_CACHE["nc"] = build_nc(N_TOTAL // N_CORES)
    return _NC_CACHE["nc"]


def finalize(partials, countss, sqaccs):
    """Host gather/unshard: combine per-core partials into the scalar loss."""
    sums = np.zeros((C, D), np.float64)
    counts = np.zeros((C,), np.float64)
    sumsq = 0.0
    for part, cnt, sq in zip(partials, countss, sqaccs):
        sums += part.astype(np.float64).reshape(C, -1, D).sum(axis=1)
        counts += cnt.astype(np.float64).reshape(-1, C).sum(axis=0)
        sumsq += float(sq.astype(np.float64).sum())
    centers = (
        np.where(counts[:, None] > 0, sums / np.maximum(counts, 1.0)[:, None], 0.0)
        + 1e-6
    )
    intra = (
        sumsq
        - 2.0 * float((sums * centers).sum())
        + float((counts * (centers**2).sum(axis=1)).sum())
    )
    cmean = centers.mean(axis=0, keepdims=True)
    inter = float(((centers - cmean) ** 2).sum()) / C
    loss = intra / (inter + 1e-6)
    return np.array(loss, dtype=np.float32)


def kernel(features: np.ndarray, labels: np.ndarray) -> np.ndarray:
    features = np.asarray(features)
    labels = np.asarray(labels)
    assert features.shape == (N_TOTAL, D), features.shape
    assert labels.shape == (N_TOTAL, C), labels.shape
    nc = _get_nc()
    rows = N_TOTAL // N_CORES
    in_maps = []
    for i in range(N_CORES):
        sl = slice(i * rows, (i + 1) * rows)
        in_maps.append(
            {
                "features": np.ascontiguousarray(features[sl], dtype=np.float32),
                "labels": np.ascontiguousarray(labels[sl], dtype=np.float32),
            }
        )
    res = run_bass_kernel_spmd(nc, in_maps, list(range(N_CORES)))
    return finalize(
        [r["partial"] for r in res.results],
        [r["counts"] for r in res.results],
        [r["sqacc"] for r in res.results],
    )
